# revision 1
# baseline (speedup 1.0000x reference)
"""Trainium2 Bass kernel for nn_CrossAttentionLayer (4-stream cross attention).

kernel(**inputs) takes FULL unsharded inputs (keyed as in setup_inputs) and
returns the full output (tuple of 4 arrays, like the reference). Batch (8) is
sharded 1 element per NeuronCore across 8 cores (pure data parallel).

Geometry per core, with C=512, L=256, H=W=64, N=4096:
  The reference's raw .view on the [L,H,W] conv output re-interprets it as
  [H,W,L]; since L=4*64, token t=(l,b) (l=0..255 conv channel, b=0..15)
  has feature vector y[l, b*256 : (b+1)*256] -- a CONTIGUOUS 256-pixel run
  of row l. Output pixel n = l*16 + b corresponds 1:1 to token (l,b).

  So in the natural [L(part), N(free)] layout, a [128, 256] slice is 128
  tokens x 256 features: attention scalars (sim, softmax, attn) are
  per-partition values -- no cross-partition work anywhere.

Pipeline per core:
  phase1 (per 512-px chunk, per stream): DMA x chunk; fp32r matmuls for
    k|q (BN scale folded into weights host-side) and v; ACT evacuates PSUM
    with relu+bias (k,q) / bias (v) as bf16 chunk tiles.
  attention (per chunk): sim via fused scalar_tensor_tensor per (pair,
    l-tile, b); one-shot strided-AP softmax over the 4 k-streams; ctx via
    scalar_tensor_tensor chains with per-partition attn scalars; PE
    transposes ctx into rhs[f, b*256+l] (b-major, contiguous copies).
  phase2 (per 512-token block): up-projection matmul reading rhs through a
    strided (l-outer, b-inner) AP so PSUM columns land in output-pixel
    order, plus an fp32r identity matmul adding the residual x; bias via
    copy-with-bias; batched 1 MiB output DMA.
"""

import numpy as np

import concourse.bass as bass
import concourse.bacc as bacc
import concourse.mybir as mybir
from concourse.tile import TileContext
from concourse.bass_utils import run_bass_kernel_spmd

B, C, L, HW = 8, 512, 256, 64
N = HW * HW              # 4096 pixels
F = 256                  # token feature length (= N // 16)
NB = N // F              # 16 b-blocks
EPS = 1e-5
NCORES = 8
CHUNK = 512              # pixel chunk (2 b-blocks)
NCHUNKS = N // CHUNK     # 8
CC = C // 128            # 4 contraction chunks
LT = L // 128            # 2 l-tiles

FP32 = mybir.dt.float32
FP32R = mybir.dt.float32r
BF16 = mybir.dt.bfloat16
AF = mybir.ActivationFunctionType
ALU = mybir.AluOpType

_cached = {}


def _build_program(loop_iters=None):
    nc = bacc.Bacc("TRN2", target_bir_lowering=False, debug=False)

    xs = [nc.declare_dram_parameter(f"x{s}", [C, N], FP32R, isOutput=False)
          for s in range(4)]
    # host-prearranged weight images (exact SBUF layouts)
    wkq_d = nc.declare_dram_parameter("wkq", [128, 4 * 4 * 4 * 128], FP32R, isOutput=False)
    wd_d = nc.declare_dram_parameter("wd", [128, 4 * 2 * 4 * 128], FP32R, isOutput=False)
    wu_d = nc.declare_dram_parameter("wu", [128, 4 * 2 * 4 * 128], BF16, isOutput=False)
    bkq_d = nc.declare_dram_parameter("bkq", [128, 16], FP32, isOutput=False)
    bd_d = nc.declare_dram_parameter("bd", [128, 8], FP32, isOutput=False)
    bu_d = nc.declare_dram_parameter("bu", [128, 16], FP32, isOutput=False)
    id_d = nc.declare_dram_parameter("ident", [128, 128], FP32R, isOutput=False)
    idb_d = nc.declare_dram_parameter("identb", [128, 128], BF16, isOutput=False)
    os_ = [nc.declare_dram_parameter(f"o{s}", [C, N], FP32, isOutput=True)
           for s in range(4)]

    with TileContext(nc) as tc:
        with (
            tc.tile_pool(name="wpool", bufs=1) as wpool,
            tc.tile_pool(name="xpool", bufs=2) as xpool,
            tc.tile_pool(name="kqvp", bufs=1) as kqvp,
            tc.tile_pool(name="attp", bufs=1) as attp,
            tc.tile_pool(name="rhsp", bufs=1) as rhsp,
            tc.tile_pool(name="outp", bufs=1) as outp,
            tc.tile_pool(name="ps_c", bufs=4, space="PSUM") as ps_c,
            tc.tile_pool(name="ps_t", bufs=2, space="PSUM") as ps_t,
            tc.tile_pool(name="ps_up", bufs=2, space="PSUM") as ps_up,
        ):
            # ---- weights ----
            wkq = wpool.tile([128, 16, 4, 128], FP32R)   # [c, (s,mc), j, m]
            nc.sync.dma_start(out=wkq[:], in_=wkq_d.ap().rearrange(
                "p (a j m) -> p a j m", a=16, j=4))
            wd = wpool.tile([128, 8, 4, 128], FP32R)     # [c, (s,lt), j, m]
            nc.sync.dma_start(out=wd[:], in_=wd_d.ap().rearrange(
                "p (a j m) -> p a j m", a=8, j=4))
            wu = wpool.tile([128, 8, 4, 128], BF16)     # [f, (s,fh), j, c]
            nc.sync.dma_start(out=wu[:], in_=wu_d.ap().rearrange(
                "p (a j m) -> p a j m", a=8, j=4))
            bkq = wpool.tile([128, 16], FP32)
            nc.sync.dma_start(out=bkq[:], in_=bkq_d.ap())
            bd = wpool.tile([128, 8], FP32)
            nc.sync.dma_start(out=bd[:], in_=bd_d.ap())
            bu = wpool.tile([128, 16], FP32)
            nc.sync.dma_start(out=bu[:], in_=bu_d.ap())
            ident = wpool.tile([128, 128], FP32R)
            nc.sync.dma_start(out=ident[:], in_=id_d.ap())
            identb = wpool.tile([128, 128], BF16)
            nc.sync.dma_start(out=identb[:], in_=idb_d.ap())

            # rhs: transposed ctx, per stream 2 f-half tiles; column layout
            # b*256 + lt*128 + l_local (b-major -> contiguous writes).
            rhs = rhsp.tile([128, 4, 2, N], BF16)  # [f_local, s, fh, col]

            def _body():
                for ci in range(NCHUNKS):
                    n0 = ci * CHUNK
                    kch, qch, vch = [], [], []
                    for s in range(4):
                        xt = xpool.tile([128, CC, CHUNK], FP32R, tag="x", name="xt")
                        nc.sync.dma_start(
                            out=xt[:],
                            in_=xs[s].ap().rearrange("(j p) n -> p j n", p=128)[:, :, n0:n0 + CHUNK])
                        kc = kqvp.tile([128, LT, CHUNK], BF16, tag=f"k{s}", name=f"kc{s}")
                        qc = kqvp.tile([128, LT, CHUNK], BF16, tag=f"q{s}", name=f"qc{s}")
                        vc = kqvp.tile([128, LT, CHUNK], BF16, tag=f"v{s}", name=f"vc{s}")
                        # k|q: mc 0,1 = k l-tiles; 2,3 = q l-tiles
                        for mc in range(4):
                            pcv = ps_c.tile([128, CHUNK], FP32, tag="conv", name="pcv")
                            for j in range(CC):
                                nc.tensor.matmul(
                                    out=pcv[:], lhsT=wkq[:, s * 4 + mc, j, :],
                                    rhs=xt[:, j, :],
                                    start=(j == 0), stop=(j == CC - 1))
                            dst = (kc if mc < 2 else qc)[:, mc % 2, :]
                            nc.scalar.activation(
                                out=dst, in_=pcv[:], func=AF.Relu,
                                bias=bkq[:, s * 4 + mc:s * 4 + mc + 1], scale=1.0)
                        for mc in range(2):
                            pcv = ps_c.tile([128, CHUNK], FP32, tag="conv", name="pcv2")
                            for j in range(CC):
                                nc.tensor.matmul(
                                    out=pcv[:], lhsT=wd[:, s * 2 + mc, j, :],
                                    rhs=xt[:, j, :],
                                    start=(j == 0), stop=(j == CC - 1))
                            nc.scalar.activation(
                                out=vc[:, mc, :], in_=pcv[:], func=AF.Identity,
                                bias=bd[:, s * 2 + mc:s * 2 + mc + 1], scale=1.0)
                        kch.append(kc)
                        qch.append(qc)
                        vch.append(vc)

                    # ---- attention for this chunk (2 b-blocks) ----
                    # sims[l_local, s, s', lt, b] fp32
                    sims = attp.tile([128, 4, 4, LT, 2], FP32, tag="sims", name="sims")
                    scr = attp.tile([128, F], BF16, tag="scr", name="scr")
                    for s in range(4):
                        for s2 in range(4):
                            for lt in range(LT):
                                for b in range(2):
                                    nc.vector.scalar_tensor_tensor(
                                        out=scr[:],
                                        in0=qch[s][:, lt, b * F:(b + 1) * F],
                                        scalar=0.0625,
                                        in1=kch[s2][:, lt, b * F:(b + 1) * F],
                                        op0=ALU.mult, op1=ALU.mult,
                                        accum_out=sims[:, s, s2, lt, b:b + 1])
                    # softmax over s' (axis 2): strided views
                    mx = attp.tile([128, 4, LT, 2], FP32, tag="mx", name="mx")
                    nc.vector.tensor_reduce(
                        out=mx[:], in_=sims.rearrange("p s t l b -> p s l b t"),
                        axis=mybir.AxisListType.X, op=ALU.max)
                    ex = attp.tile([128, 4, 4, LT, 2], FP32, tag="ex", name="ex")
                    nc.vector.tensor_tensor(
                        out=ex[:], in0=sims[:],
                        in1=mx.rearrange("p s l b -> p s () l b").broadcast_to((128, 4, 4, LT, 2)),
                        op=ALU.subtract)
                    nc.scalar.activation(out=ex[:], in_=ex[:], func=AF.Exp,
                                         bias=0.0, scale=1.0)
                    sm = attp.tile([128, 4, LT, 2], FP32, tag="sm", name="sm")
                    nc.vector.tensor_reduce(
                        out=sm[:], in_=ex.rearrange("p s t l b -> p s l b t"),
                        axis=mybir.AxisListType.X, op=ALU.add)
                    nc.vector.reciprocal(out=sm[:], in_=sm[:])
                    att = attp.tile([128, 4, 4, LT, 2], FP32, tag="att", name="att")
                    nc.vector.tensor_tensor(
                        out=att[:], in0=ex[:],
                        in1=sm.rearrange("p s l b -> p s () l b").broadcast_to((128, 4, 4, LT, 2)),
                        op=ALU.mult)

                    # ---- ctx + transpose into rhs ----
                    for s in range(4):
                        ctx = attp.tile([128, LT, CHUNK], BF16, tag="ctx", name="ctx")
                        for lt in range(LT):
                            for b in range(2):
                                sl = slice(b * F, (b + 1) * F)
                                nc.vector.tensor_scalar_mul(
                                    out=ctx[:, lt, sl], in0=vch[0][:, lt, sl],
                                    scalar1=att[:, s, 0, lt, b:b + 1])
                                for s2 in range(1, 4):
                                    nc.vector.scalar_tensor_tensor(
                                        out=ctx[:, lt, sl], in0=vch[s2][:, lt, sl],
                                        scalar=att[:, s, s2, lt, b:b + 1],
                                        in1=ctx[:, lt, sl],
                                        op0=ALU.mult, op1=ALU.add)
                        for lt in range(LT):
                            for b in range(2):
                                bg = 2 * ci + b   # global b index
                                for fh in range(2):
                                    pst = ps_t.tile([128, 128], BF16, tag="pst", name="pst")
                                    nc.tensor.transpose(
                                        out=pst[:],
                                        in_=ctx[:, lt, b * F + fh * 128: b * F + (fh + 1) * 128],
                                        identity=identb[:])
                                    dst = rhs[:, s, fh, bg * 256 + lt * 128: bg * 256 + (lt + 1) * 128]
                                    if (lt + b) % 2 == 0:
                                        nc.vector.tensor_copy(dst, pst[:])
                                    else:
                                        nc.scalar.copy(out=dst, in_=pst[:])

                # ================= phase 2 =================
                for s in range(4):
                    for nb in range(NCHUNKS):
                        n0 = nb * CHUNK
                        lt, lo = nb // 4, (nb % 4) * 32
                        xt = xpool.tile([128, CC, CHUNK], FP32R, tag="x", name="xt2")
                        nc.sync.dma_start(
                            out=xt[:],
                            in_=xs[s].ap().rearrange("(j p) n -> p j n", p=128)[:, :, n0:n0 + CHUNK])
                        ot = outp.tile([128, CC, CHUNK], FP32, tag="ot", name="ot")
                        for j in range(CC):
                            pup = ps_up.tile([128, CHUNK], FP32, tag="up", name="pup")
                            for fh in range(2):
                                # rhs columns gathered l-outer, b-inner so psum
                                # columns are output-pixel order n = l*16 + b
                                rap = rhs[:, s, fh, :].rearrange(
                                    "p (b q) -> p b q", q=256)[:, :, lt * 128 + lo: lt * 128 + lo + 32]
                                rap = rap.rearrange("p b l -> p l b")
                                nc.tensor.matmul(
                                    out=pup[:], lhsT=wu[:, s * 2 + fh, j, :],
                                    rhs=rap, start=(fh == 0), stop=False)
                            nc.tensor.matmul(
                                out=pup[:], lhsT=ident[:],
                                rhs=xt[:, j, :],
                                start=False, stop=True)
                            if j % 2 == 0:
                                nc.vector.tensor_scalar_add(
                                    out=ot[:, j, :], in0=pup[:],
                                    scalar1=bu[:, s * 4 + j:s * 4 + j + 1])
                            else:
                                nc.scalar.activation(
                                    out=ot[:, j, :], in_=pup[:], func=AF.Identity,
                                    bias=bu[:, s * 4 + j:s * 4 + j + 1], scale=1.0)
                        nc.sync.dma_start(
                            out=os_[s].ap().rearrange("(j p) n -> p j n", p=128)[:, :, n0:n0 + CHUNK],
                            in_=ot[:])

            if loop_iters is None:
                _body()
            else:
                with tc.For_i(0, loop_iters, 1):
                    _body()

    nc.compile()
    return nc


def _prep_weights(inputs):
    """Fold BN into conv weights host-side; produce exact SBUF images."""
    import ml_dtypes
    f32 = np.float32
    g = {k: np.asarray(v, f32) for k, v in inputs.items()}
    sk = g["gk"] / np.sqrt(g["vk"] + EPS)            # [4, L]
    sq = g["gq"] / np.sqrt(g["vq"] + EPS)
    Wk_f = g["Wk"] * sk[:, :, None]                  # [4, L, C]
    Wq_f = g["Wq"] * sq[:, :, None]
    bk_f = (g["bk"] - g["mk"]) * sk + g["betak"]     # [4, L]
    bq_f = (g["bq"] - g["mq"]) * sq + g["betaq"]

    # wkq image [c_local, (s, mc), j, m]: lhsT chunks of [Wk_f|Wq_f]^T
    wkq = np.zeros((128, 16, 4, 128), f32)
    wdv = np.zeros((128, 8, 4, 128), f32)
    wuv = np.zeros((128, 8, 4, 128), f32)
    for s in range(4):
        Wcat = np.concatenate([Wk_f[s], Wq_f[s]], axis=0)  # [512 (kq-l), C]
        for mc in range(4):
            for j in range(CC):
                wkq[:, s * 4 + mc, j, :] = \
                    Wcat[mc * 128:(mc + 1) * 128, j * 128:(j + 1) * 128].T
        for mc in range(2):
            for j in range(CC):
                wdv[:, s * 2 + mc, j, :] = \
                    g["Wd"][s][mc * 128:(mc + 1) * 128, j * 128:(j + 1) * 128].T
        # wu: lhsT[f, c] = Wu[s].T ; [f_local, (s, fh), j, c_local]
        WuT = g["Wu"][s].T                           # [L=256 (f), C]
        for fh in range(2):
            for j in range(CC):
                wuv[:, s * 2 + fh, j, :] = \
                    WuT[fh * 128:(fh + 1) * 128, j * 128:(j + 1) * 128]

    bkq = np.zeros((128, 16), f32)
    bdv = np.zeros((128, 8), f32)
    buv = np.zeros((128, 16), f32)
    for s in range(4):
        for mc in range(4):
            src = bk_f[s] if mc < 2 else bq_f[s]
            bkq[:, s * 4 + mc] = src[(mc % 2) * 128:(mc % 2) * 128 + 128]
        for mc in range(2):
            bdv[:, s * 2 + mc] = g["bd"][s][mc * 128:(mc + 1) * 128]
        for j in range(CC):
            buv[:, s * 4 + j] = g["bu"][s][j * 128:(j + 1) * 128]
    ident = np.eye(128, dtype=f32)
    return {
        "wkq": wkq.reshape(128, -1), "wd": wdv.reshape(128, -1),
        "wu": wuv.reshape(128, -1).astype(ml_dtypes.bfloat16),
        "bkq": bkq, "bd": bdv, "bu": buv,
        "ident": ident, "identb": ident.astype(ml_dtypes.bfloat16),
    }


def get_program(loop_iters=None):
    key = ("nc", loop_iters)
    if key not in _cached:
        _cached[key] = _build_program(loop_iters)
    return _cached[key]


def make_in_maps(inputs):
    w = _prep_weights(inputs)
    names = ("x_f", "x_g", "x_h", "x_t")
    xs = {nm: np.asarray(inputs[nm], np.float32).reshape(B, C, N) for nm in names}
    in_maps = []
    for b in range(B):
        m = dict(w)
        for s, nm in enumerate(names):
            m[f"x{s}"] = np.ascontiguousarray(xs[nm][b])
        in_maps.append(m)
    return in_maps


def kernel(**inputs):
    nc = get_program()
    in_maps = make_in_maps(inputs)
    res = run_bass_kernel_spmd(nc, in_maps, core_ids=list(range(NCORES)))
    outs = []
    for s in range(4):
        o = np.stack([res.results[b][f"o{s}"] for b in range(B)], axis=0)
        outs.append(o.reshape(B, C, HW, HW))
    return tuple(outs)



# revision 16
# speedup vs baseline: 1.3783x; 1.3783x over previous
"""Trainium2 Bass kernel for nn_CrossAttentionLayer (4-stream cross attention).

kernel(**inputs) takes FULL unsharded inputs (keyed as in setup_inputs) and
returns the full output (tuple of 4 arrays). Batch (8) is sharded 1 element
per NeuronCore across 8 cores (pure data parallel).

v2 design (vs the fp32r baseline):
  - x shipped twice host-side: fp8e4 copy (conv inputs) + bf16 copy with the
    up-projection bias bu pre-added (residual stream). Output written bf16.
    HBM traffic per core: 8 + 16 + 16 = 40 MiB (baseline 96 MiB fp32).
  - All three 1x1 convs and the up-projection run as fp8 DoubleRow matmuls
    (contract 256 per instruction, 2x bf16 rate). BN scale folded into Wk/Wq
    host-side; all fp8 weights scaled x16 to avoid e4m3 subnormals, undone
    in the PSUM evacuation (ACT scale=1/16) or via a 16*I residual matmul.
  - Residual: psum = 16*up + 16*(x+bu) via a 16*I bf16 identity matmul in the
    same PSUM accumulation group; evacuation is a pure Copy with scale 1/16.
  - Token geometry as baseline: the reference's raw .view makes token (l,b)
    have features = a contiguous 256-px run of conv row l, so attention
    scalars are per-partition values; sim/softmax/ctx run on DVE with
    bf16-packed operands (fast DVE modes), ctx transposed via PE into rhs.
  - Evacuations spread across ACT (k|q), Pool/gpsimd (v), DVE (transpose
    copies), ACT+Pool (phase-2) to balance engine busy time.
"""

import numpy as np

import concourse.bass as bass
import concourse.bacc as bacc
import concourse.mybir as mybir
from concourse.tile import TileContext
from concourse.bass_utils import run_bass_kernel_spmd

B, C, L, HW = 8, 512, 256, 64
N = HW * HW              # 4096 pixels
F = 256                  # token feature length (= N // 16)
NB = N // F              # 16 b-blocks
EPS = 1e-5
NCORES = 8
CHUNK = 512              # pixel chunk (2 b-blocks)
NCHUNKS = N // CHUNK     # 8
CC = C // 128            # 4 contraction chunks
LT = L // 128            # 2 l-tiles
WS = 16.0                # fp8 weight scale
WSI = 1.0 / WS

FP32 = mybir.dt.float32
BF16 = mybir.dt.bfloat16
FP8 = mybir.dt.float8e4
AF = mybir.ActivationFunctionType
ALU = mybir.AluOpType
DR = mybir.MatmulPerfMode.DoubleRow

_cached = {}


def _build_program(loop_iters=None):
    nc = bacc.Bacc("TRN2", target_bir_lowering=False, debug=False)

    # x8[p, ci, s, j, q] = x_s[j*128+p, ci*512+q]      (conv input, fp8)
    x8_d = nc.declare_dram_parameter("x8", [128, NCHUNKS, 4, CC, CHUNK], FP8,
                                     isOutput=False)
    # xb[p, nb, s, j, q] = x_s[j*128+p, nb*512+q] + bu_s[j*128+p]
    xb_d = nc.declare_dram_parameter("xb", [128, NCHUNKS, 4, CC, CHUNK], BF16,
                                     isOutput=False)
    # wkq8[p, s*4+mc, g, t, m] = 16*Wcat_s[mc*128+m, (2g+t)*128+p]
    wkq_d = nc.declare_dram_parameter("wkq", [128, 16 * 2 * 2 * 128], FP8,
                                      isOutput=False)
    wd_d = nc.declare_dram_parameter("wd", [128, 8 * 2 * 2 * 128], FP8,
                                     isOutput=False)
    # wu8[p, s, j, t, m] = 16*Wu_s[j*128+m, t*128+p]
    wu_d = nc.declare_dram_parameter("wu", [128, 4 * 4 * 2 * 128], FP8,
                                     isOutput=False)
    # biases as fp8 K=1 matmul weights: row vectors [1, 16*128] / [1, 8*128]
    bkq_d = nc.declare_dram_parameter("bkq", [1, 16 * 128], FP8, isOutput=False)
    bd_d = nc.declare_dram_parameter("bd", [1, 8 * 128], FP8, isOutput=False)
    ones_d = nc.declare_dram_parameter("ones8", [1, CHUNK], FP8, isOutput=False)
    idb_d = nc.declare_dram_parameter("identb", [128, 128], BF16, isOutput=False)
    id16_d = nc.declare_dram_parameter("ident16", [128, 128], BF16, isOutput=False)
    # o[p, nb, s, j, q] = out_s[j*128+p, nb*512+q]   (bf16)
    o_d = nc.declare_dram_parameter("o", [128, NCHUNKS, 4, CC, CHUNK], BF16,
                                    isOutput=True)

    with TileContext(nc) as tc:
        with (
            tc.tile_pool(name="wpool", bufs=1) as wpool,
            tc.tile_pool(name="xpool", bufs=2) as xpool,
            tc.tile_pool(name="xbp", bufs=2) as xbp,
            tc.tile_pool(name="kqvp", bufs=2) as kqvp,
            tc.tile_pool(name="attp", bufs=2) as attp,
            tc.tile_pool(name="rhsp", bufs=1) as rhsp,
            tc.tile_pool(name="outp", bufs=2) as outp,
            tc.tile_pool(name="ps_c", bufs=3, space="PSUM") as ps_c,
            tc.tile_pool(name="ps_t", bufs=2, space="PSUM") as ps_t,
        ):
            # ---- weights ----
            wkq = wpool.tile([128, 16, 2, 2, 128], FP8)
            nc.sync.dma_start(out=wkq[:], in_=wkq_d.ap().rearrange(
                "p (a g t m) -> p a g t m", a=16, g=2, t=2))
            wd = wpool.tile([128, 8, 2, 2, 128], FP8)
            nc.sync.dma_start(out=wd[:], in_=wd_d.ap().rearrange(
                "p (a g t m) -> p a g t m", a=8, g=2, t=2))
            wu = wpool.tile([128, 4, 4, 2, 128], FP8)
            nc.sync.dma_start(out=wu[:], in_=wu_d.ap().rearrange(
                "p (s j t m) -> p s j t m", s=4, j=4, t=2))
            bkq = wpool.tile([128, 16], FP32)
            nc.sync.dma_start(out=bkq[:], in_=bkq_d.ap())
            bd = wpool.tile([128, 8], FP32)
            nc.sync.dma_start(out=bd[:], in_=bd_d.ap())
            identb = wpool.tile([128, 128], BF16)
            nc.sync.dma_start(out=identb[:], in_=idb_d.ap())
            ident16 = wpool.tile([128, 128], BF16)
            nc.sync.dma_start(out=ident16[:], in_=id16_d.ap())

            # rhs: transposed ctx; columns already in output-pixel order
            # col = l*16 + b so phase 2 reads a contiguous slice.
            # fp8 so the phase-2 up-projection can run in DoubleRow mode.
            rhs = rhsp.tile([128, 4, 2, N], FP8)  # [f_local, s, fh, col]

            def _body():
                for ci in range(NCHUNKS):
                    kch, qch, vch = [], [], []
                    xt4 = xpool.tile([128, 4, CC, CHUNK], FP8, tag="x", name="xt")
                    nc.sync.dma_start(out=xt4[:], in_=x8_d.ap()[:, ci])
                    for s in range(4):
                        xt = xt4[:, s]
                        kc = kqvp.tile([128, LT, CHUNK], BF16, tag=f"k{s}", name=f"kc{s}")
                        qc = kqvp.tile([128, LT, CHUNK], BF16, tag=f"q{s}", name=f"qc{s}")
                        vc = kqvp.tile([128, LT, CHUNK], BF16, tag=f"v{s}", name=f"vc{s}")
                        # k: mc 0,1  q: mc 2,3 (each = one l-tile)
                        for pair, dst in ((0, kc), (1, qc)):
                            pcv = ps_c.tile([128, 2, CHUNK], FP32, tag="conv", name="pcv")
                            for lt in range(LT):
                                mc = pair * 2 + lt
                                for g in range(2):
                                    nc.tensor.matmul(
                                        out=pcv[:, lt, :],
                                        lhsT=wkq[:, s * 4 + mc, g],
                                        rhs=xt[:, 2 * g:2 * g + 2, :],
                                        start=(g == 0), stop=(g == 1),
                                        perf_mode=DR)
                            for lt in range(LT):
                                mc = pair * 2 + lt
                                nc.scalar.activation(
                                    out=dst[:, lt, :], in_=pcv[:, lt, :],
                                    func=AF.Relu,
                                    bias=bkq[:, s * 4 + mc:s * 4 + mc + 1],
                                    scale=WSI)
                        pcv = ps_c.tile([128, 2, CHUNK], FP32, tag="conv", name="pcv2")
                        for lt in range(LT):
                            for g in range(2):
                                nc.tensor.matmul(
                                    out=pcv[:, lt, :],
                                    lhsT=wd[:, s * 2 + lt, g],
                                    rhs=xt[:, 2 * g:2 * g + 2, :],
                                    start=(g == 0), stop=(g == 1),
                                    perf_mode=DR)
                        for lt in range(LT):
                            nc.vector.tensor_scalar(
                                out=vc[:, lt, :], in0=pcv[:, lt, :],
                                scalar1=WSI,
                                scalar2=bd[:, s * 2 + lt:s * 2 + lt + 1],
                                op0=ALU.mult, op1=ALU.add)
                        kch.append(kc)
                        qch.append(qc)
                        vch.append(vc)

                    # ---- attention for this chunk (2 b-blocks) ----
                    sims = attp.tile([128, 4, 4, LT, 2], FP32, tag="sims", name="sims")
                    scr = attp.tile([128, F], BF16, tag="scr", name="scr")
                    for s in range(4):
                        for s2 in range(4):
                            for lt in range(LT):
                                for b in range(2):
                                    nc.vector.scalar_tensor_tensor(
                                        out=scr[:],
                                        in0=qch[s][:, lt, b * F:(b + 1) * F],
                                        scalar=0.0625,
                                        in1=kch[s2][:, lt, b * F:(b + 1) * F],
                                        op0=ALU.mult, op1=ALU.mult,
                                        accum_out=sims[:, s, s2, lt, b:b + 1])
                    # softmax over s' (axis 2): strided views
                    mx = attp.tile([128, 4, LT, 2], FP32, tag="mx", name="mx")
                    nc.vector.tensor_reduce(
                        out=mx[:], in_=sims.rearrange("p s t l b -> p s l b t"),
                        axis=mybir.AxisListType.X, op=ALU.max)
                    ex = attp.tile([128, 4, 4, LT, 2], FP32, tag="ex", name="ex")
                    nc.vector.tensor_tensor(
                        out=ex[:], in0=sims[:],
                        in1=mx.rearrange("p s l b -> p s () l b").broadcast_to((128, 4, 4, LT, 2)),
                        op=ALU.subtract)
                    nc.scalar.activation(out=ex[:], in_=ex[:], func=AF.Exp,
                                         bias=0.0, scale=1.0)
                    sm = attp.tile([128, 4, LT, 2], FP32, tag="sm", name="sm")
                    nc.vector.tensor_reduce(
                        out=sm[:], in_=ex.rearrange("p s t l b -> p s l b t"),
                        axis=mybir.AxisListType.X, op=ALU.add)
                    nc.vector.reciprocal(out=sm[:], in_=sm[:])
                    att = attp.tile([128, 4, 4, LT, 2], FP32, tag="att", name="att")
                    nc.vector.tensor_tensor(
                        out=att[:], in0=ex[:],
                        in1=sm.rearrange("p s l b -> p s () l b").broadcast_to((128, 4, 4, LT, 2)),
                        op=ALU.mult)

                    # ---- ctx + transpose into rhs ----
                    for s in range(4):
                        ctx = attp.tile([128, LT, CHUNK], BF16, tag=f"ctx{s % 2}",
                                        name=f"ctx{s}")
                        for lt in range(LT):
                            for b in range(2):
                                sl = slice(b * F, (b + 1) * F)
                                nc.vector.tensor_scalar_mul(
                                    out=ctx[:, lt, sl], in0=vch[0][:, lt, sl],
                                    scalar1=att[:, s, 0, lt, b:b + 1])
                                for s2 in range(1, 4):
                                    nc.vector.scalar_tensor_tensor(
                                        out=ctx[:, lt, sl], in0=vch[s2][:, lt, sl],
                                        scalar=att[:, s, s2, lt, b:b + 1],
                                        in1=ctx[:, lt, sl],
                                        op0=ALU.mult, op1=ALU.add)
                        for b in range(2):
                            bg = 2 * ci + b   # global b index
                            pst = ps_t.tile([128, 2, 2, 128], BF16, tag="pst", name="pst")
                            for fh in range(2):
                                for lt in range(LT):
                                    nc.tensor.transpose(
                                        out=pst[:, fh, lt, :],
                                        in_=ctx[:, lt, b * F + fh * 128: b * F + (fh + 1) * 128],
                                        identity=identb[:])
                            # scatter into pixel-ordered columns: col = (lt*128+m)*16 + bg
                            dst = rhs[:, s, :, :].rearrange(
                                "p f (l2 m b2) -> p f l2 m b2", l2=2, m=128, b2=16)[:, :, :, :, bg]
                            nc.vector.tensor_copy(dst, pst[:])

                # ================= phase 2 =================
                for nb in range(NCHUNKS):
                    xbt4 = xbp.tile([128, 4, CC, CHUNK], BF16, tag="xb", name="xbt")
                    nc.sync.dma_start(out=xbt4[:], in_=xb_d.ap()[:, nb])
                    ot4 = outp.tile([128, 4, CC, CHUNK], BF16, tag="ot", name="ot")
                    for s in range(4):
                        # rhs columns are already in output-pixel order
                        rap = rhs[:, s, :, nb * CHUNK:(nb + 1) * CHUNK]
                        for jj in range(2):
                            pup = ps_c.tile([128, 2, CHUNK], FP32, tag="conv", name="pup")
                            for j2 in range(2):
                                j = jj * 2 + j2
                                nc.tensor.matmul(
                                    out=pup[:, j2, :], lhsT=wu[:, s, j],
                                    rhs=rap, start=True, stop=False,
                                    perf_mode=DR)
                                nc.tensor.matmul(
                                    out=pup[:, j2, :], lhsT=ident16[:],
                                    rhs=xbt4[:, s, j, :],
                                    start=False, stop=True)
                            dst = ot4[:, s, jj * 2:jj * 2 + 2, :]
                            if jj == 0:
                                nc.scalar.activation(
                                    out=dst, in_=pup[:], func=AF.Copy,
                                    bias=0.0, scale=WSI)
                            else:
                                nc.vector.tensor_scalar_mul(
                                    out=dst, in0=pup[:], scalar1=WSI)
                    nc.sync.dma_start(out=o_d.ap()[:, nb], in_=ot4[:])

            if loop_iters is None:
                _body()
            else:
                with tc.For_i(0, loop_iters, 1):
                    _body()

    nc.compile()
    return nc


def _prep_weights(inputs):
    """Fold BN into conv weights host-side; produce exact SBUF images."""
    import ml_dtypes
    f32 = np.float32
    fp8 = ml_dtypes.float8_e4m3
    bf16 = ml_dtypes.bfloat16
    g = {k: np.asarray(v, f32) for k, v in inputs.items()}
    sk = g["gk"] / np.sqrt(g["vk"] + EPS)            # [4, L]
    sq = g["gq"] / np.sqrt(g["vq"] + EPS)
    Wk_f = g["Wk"] * sk[:, :, None]                  # [4, L, C]
    Wq_f = g["Wq"] * sq[:, :, None]
    bk_f = (g["bk"] - g["mk"]) * sk + g["betak"]     # [4, L]
    bq_f = (g["bq"] - g["mq"]) * sq + g["betaq"]

    wkq = np.zeros((128, 16, 2, 2, 128), f32)
    wdv = np.zeros((128, 8, 2, 2, 128), f32)
    wuv = np.zeros((128, 4, 4, 2, 128), f32)
    for s in range(4):
        Wcat = np.concatenate([Wk_f[s], Wq_f[s]], axis=0)  # [512 (kq-l), C]
        for mc in range(4):
            for gg in range(2):
                for t in range(2):
                    wkq[:, s * 4 + mc, gg, t, :] = WS * \
                        Wcat[mc * 128:(mc + 1) * 128,
                             (2 * gg + t) * 128:(2 * gg + t + 1) * 128].T
        for mc in range(2):
            for gg in range(2):
                for t in range(2):
                    wdv[:, s * 2 + mc, gg, t, :] = WS * \
                        g["Wd"][s][mc * 128:(mc + 1) * 128,
                                   (2 * gg + t) * 128:(2 * gg + t + 1) * 128].T
        for j in range(CC):
            for t in range(2):
                # wu8[p, s, j, t, m] = 16*Wu_s[j*128+m, t*128+p]
                wuv[:, s, j, t, :] = WS * \
                    g["Wu"][s][j * 128:(j + 1) * 128,
                               t * 128:(t + 1) * 128].T

    bkq = np.zeros((128, 16), f32)
    bdv = np.zeros((128, 8), f32)
    for s in range(4):
        for mc in range(4):
            src = bk_f[s] if mc < 2 else bq_f[s]
            bkq[:, s * 4 + mc] = src[(mc % 2) * 128:(mc % 2) * 128 + 128]
        for mc in range(2):
            bdv[:, s * 2 + mc] = g["bd"][s][mc * 128:(mc + 1) * 128]
    ident = np.eye(128, dtype=f32)
    return {
        "wkq": wkq.reshape(128, -1).astype(fp8),
        "wd": wdv.reshape(128, -1).astype(fp8),
        "wu": wuv.reshape(128, -1).astype(fp8),
        "bkq": bkq, "bd": bdv,
        "identb": ident.astype(bf16),
        "ident16": (16.0 * ident).astype(bf16),
    }


def get_program(loop_iters=None):
    key = ("nc", loop_iters)
    if key not in _cached:
        _cached[key] = _build_program(loop_iters)
    return _cached[key]


def make_in_maps(inputs):
    import ml_dtypes
    fp8 = ml_dtypes.float8_e4m3
    bf16 = ml_dtypes.bfloat16
    w = _prep_weights(inputs)
    names = ("x_f", "x_g", "x_h", "x_t")
    bu = np.asarray(inputs["bu"], np.float32)        # [4, C]
    # [B, 4s, C, N] -> per-core views
    xs = np.stack([np.asarray(inputs[nm], np.float32).reshape(B, C, N)
                   for nm in names], axis=1)
    # x8[p, ci, s, j, q] = x_s[j*128+p, ci*512+q]
    x4 = xs.reshape(B, 4, CC, 128, NCHUNKS, CHUNK)   # [B,s,j,p,ci,q]
    x8 = x4.transpose(0, 3, 4, 1, 2, 5)              # [B,p,ci,s,j,q]
    xb4 = (xs + bu[None, :, :, None]).reshape(B, 4, CC, 128, NCHUNKS, CHUNK)
    xb = xb4.transpose(0, 3, 4, 1, 2, 5)
    x8 = np.ascontiguousarray(x8).astype(fp8)
    xb = np.ascontiguousarray(xb).astype(bf16)
    in_maps = []
    for b in range(B):
        m = dict(w)
        m["x8"] = x8[b]
        m["xb"] = xb[b]
        in_maps.append(m)
    return in_maps


def kernel(**inputs):
    nc = get_program()
    in_maps = make_in_maps(inputs)
    res = run_bass_kernel_spmd(nc, in_maps, core_ids=list(range(NCORES)))
    # o[p, nb, s, j, q] -> out_s[j*128+p, nb*512+q]
    o = np.stack([np.asarray(res.results[b]["o"], np.float32)
                  for b in range(B)], axis=0)        # [B,p,nb,s,j,q]
    o = o.transpose(3, 0, 4, 1, 2, 5)                # [s,B,j,p,nb,q]
    o = o.reshape(4, B, C, HW, HW)
    return tuple(o[s] for s in range(4))
